# revision 1
# baseline (speedup 1.0000x reference)
"""Trainium2 Bass kernel for nn_CrossModalFusionModel (sparse sliding-window
cross-attention, 2 modules: image<-text and text<-image).

Sharding: head-parallel tensor parallelism over 8 NeuronCores. Core h owns
attention head h (dh=128) of BOTH modules: it computes its head's Q/K/V with
host-folded projection weights (input-proj and attention-proj chains collapse
into one matmul), runs full-sequence banded attention for that head, and emits
a full-D o-projection partial plus its D-slice of the residual projection.
The host sums the 8 partials (the unshard step). No collectives.

Everything on-device lives in transposed [D, seq] layout so scores/AV/o-proj
chain through the PE without any on-chip transposes; the host pre-transposes
inputs and post-transposes outputs.

The reference's zero-pad attention slots (up to window tokens of kb/vb at the
sequence edges) collapse into ONE virtual pad column per query with
multiplicative weight n_pad(i), since all pad slots share the score q.kb.
"""

import math

import numpy as np
import ml_dtypes

N = 512          # tokens / patches
DM = 1024        # d_model
DH = 128         # head dim
NT = N // 128    # 4 j-tiles
C_IMG = 1024
C_TXT = 768
WINDOW = 64
NCORES = 8

# compute dtype knob: "f32r" | "f16" | "bf16" | "f32"
COMPUTE_DTYPE = "f32r"

_prog_cache = {}
LAST_RESULT = {}


def _mybir_cd(cd):
    from concourse import mybir
    return {
        "f32r": mybir.dt.float32r,
        "f16": mybir.dt.float16,
        "bf16": mybir.dt.bfloat16,
        "f32": mybir.dt.float32,
    }[cd]


def _np_cd(cd):
    return {
        "f32r": np.float32,
        "f16": np.float16,
        "bf16": ml_dtypes.bfloat16,
        "f32": np.float32,
    }[cd]


def _host_cd(x, cd):
    """Convert a float64 host array to the wire format of compute dtype cd."""
    x = np.ascontiguousarray(x)
    if cd == "f32r":
        x = x.astype(np.float32)
        hi = x.astype(ml_dtypes.bfloat16).astype(np.float32)
        lo = (x - hi).astype(ml_dtypes.bfloat16).astype(np.float32)
        return hi + lo
    return x.astype(_np_cd(cd))


def _build_program(cd):
    import concourse.bass as bass
    import concourse.tile as tile
    from concourse import bacc, mybir

    f32 = mybir.dt.float32
    CD = _mybir_cd(cd)
    Exp = mybir.ActivationFunctionType.Exp

    nc = bacc.Bacc("TRN2", target_bir_lowering=False, debug=False,
                   num_devices=NCORES)

    def din(name, shape, dt=CD):
        return nc.dram_tensor(name, shape, dt, kind="ExternalInput")

    def dout(name, shape, dt=f32):
        return nc.dram_tensor(name, shape, dt, kind="ExternalOutput")

    # Activations (transposed) and masks are identical on every core.
    xT_img = din("xT_img", [C_IMG, N])
    xT_txt = din("xT_txt", [C_TXT, N])
    maskM = din("maskM", [128, NT * N])
    maskP = din("maskP", [1, N])

    # Per-core (per-head) folded weights.
    mods = {}
    for m, cq, cc in (("ia", C_IMG, C_TXT), ("ta", C_TXT, C_IMG)):
        mods[m] = dict(
            wqT=din(f"wqT_{m}", [cq, DH]),
            wkT=din(f"wkT_{m}", [cc, DH]),
            wvT=din(f"wvT_{m}", [cc, DH]),
            woT=din(f"woT_{m}", [DH, DM]),
            kbcol=din(f"kbcol_{m}", [DH, 1]),
            vbpad=din(f"vbpad_{m}", [1, DH]),
            bvrow=din(f"bvrow_{m}", [1, DH]),
            bq=din(f"bq_{m}", [DH, 1], f32),
            bk=din(f"bk_{m}", [DH, 1], f32),
            po=dout(f"po_{m}", [DM, N]),
            xr=dout(f"xr_{m}", [DH, N]),
        )
    rwT_img = din("rwT_img", [C_IMG, DH])   # ip_w D-slice (residual)
    rwT_txt = din("rwT_txt", [C_TXT, DH])   # tp_w D-slice
    brx = din("brx", [DH, 1], f32)          # ip_b slice
    brt = din("brt", [DH, 1], f32)          # tp_b slice
    ones_c = din("ones_c", [128, 1])
    ones_r = din("ones_r", [1, 128])

    with tile.TileContext(nc) as tc:
        with tc.tile_pool(name="consts", bufs=1) as consts, \
             tc.tile_pool(name="work", bufs=3) as work, \
             tc.tile_pool(name="epool", bufs=3) as epool, \
             tc.tile_pool(name="small", bufs=2) as small, \
             tc.tile_pool(name="ps_st", bufs=2, space="PSUM") as ps_st, \
             tc.tile_pool(name="ps_small", bufs=1, space="PSUM") as ps_small, \
             tc.tile_pool(name="ps_acc", bufs=4, space="PSUM") as ps_acc:

            def load3(name, dram, c, n):
                t = consts.tile([128, c // 128, n], CD, tag=name)
                nc.sync.dma_start(
                    t[:], dram.ap().rearrange("(c p) n -> p c n", p=128))
                return t

            xi = load3("xi", xT_img, C_IMG, N)
            xt = load3("xt", xT_txt, C_TXT, N)
            rwi = load3("rwi", rwT_img, C_IMG, DH)
            rwt = load3("rwt", rwT_txt, C_TXT, DH)

            mM = consts.tile([128, NT * N], CD, tag="mM")
            nc.sync.dma_start(mM[:], maskM[:])
            mP = consts.tile([1, N], CD, tag="mP")
            nc.sync.dma_start(mP[:], maskP[:])

            ones_col = consts.tile([128, 1], CD, tag="ones_col")
            nc.sync.dma_start(ones_col[:], ones_c[:])
            ones_row = consts.tile([1, 128], CD, tag="ones_row")
            nc.sync.dma_start(ones_row[:], ones_r[:])

            sb = {}
            for m, cq, cc in (("ia", C_IMG, C_TXT), ("ta", C_TXT, C_IMG)):
                d = mods[m]
                sb[m] = dict(
                    wq=load3(f"wq_{m}", d["wqT"], cq, DH),
                    wk=load3(f"wk_{m}", d["wkT"], cc, DH),
                    wv=load3(f"wv_{m}", d["wvT"], cc, DH),
                )
                wo = consts.tile([DH, DM], CD, tag=f"wo_{m}")
                nc.sync.dma_start(wo[:], d["woT"][:])
                kbc = consts.tile([DH, 1], CD, tag=f"kbc_{m}")
                nc.sync.dma_start(kbc[:], d["kbcol"][:])
                vbp = consts.tile([1, DH], CD, tag=f"vbp_{m}")
                nc.sync.dma_start(vbp[:], d["vbpad"][:])
                bvr = consts.tile([1, DH], CD, tag=f"bvr_{m}")
                nc.sync.dma_start(bvr[:], d["bvrow"][:])
                bq = consts.tile([DH, 1], f32, tag=f"bq_{m}")
                nc.sync.dma_start(bq[:], d["bq"][:])
                bk = consts.tile([DH, 1], f32, tag=f"bk_{m}")
                nc.sync.dma_start(bk[:], d["bk"][:])
                sb[m].update(wo=wo, kbc=kbc, vbp=vbp, bvr=bvr, bq=bq, bk=bk)
            bxi = consts.tile([DH, 1], f32, tag="bxi")
            nc.sync.dma_start(bxi[:], brx[:])
            bxt = consts.tile([DH, 1], f32, tag="bxt")
            nc.sync.dma_start(bxt[:], brt[:])

            def projT(w3, x3, nct, bias_col, tag, out_dt=CD):
                """out^T [128, N] = (x @ W^T)^T + bias, via contraction tiles."""
                ps = ps_acc.tile([128, N], f32, tag="acc")
                for ct in range(nct):
                    nc.tensor.matmul(ps[:], w3[:, ct, :], x3[:, ct, :],
                                     start=(ct == 0), stop=(ct == nct - 1))
                out = work.tile([128, N], out_dt, tag="sb_" + tag)
                nc.vector.tensor_scalar_add(out[:], ps[:], bias_col[:])
                return out

            def vproj_nat(x3, w3, nct, bvr, tag):
                """V natural [j, d] in one [128, NT*128] tile (jt at free jt*128)."""
                ps = ps_acc.tile([128, NT * DH], f32, tag="acc")
                for jt in range(NT):
                    blk = ps[:, jt * DH:(jt + 1) * DH]
                    for ct in range(nct):
                        nc.tensor.matmul(
                            blk, x3[:, ct, jt * 128:(jt + 1) * 128],
                            w3[:, ct, :], start=(ct == 0), stop=False)
                    nc.tensor.matmul(blk, ones_row[:, :], bvr[:],
                                     start=False, stop=True)
                out = work.tile([128, NT * DH], CD, tag="sb_" + tag)
                nc.vector.tensor_copy(out[:], ps[:])
                return out

            def residT(w3, x3, nct, bias_col, dram, tag):
                ps = ps_acc.tile([128, N], f32, tag="acc")
                for ct in range(nct):
                    nc.tensor.matmul(ps[:], w3[:, ct, :], x3[:, ct, :],
                                     start=(ct == 0), stop=(ct == nct - 1))
                out = work.tile([128, N], f32, tag="sb_" + tag)
                nc.vector.tensor_scalar_add(out[:], ps[:], bias_col[:])
                nc.sync.dma_start(dram[:], out[:])

            for m, xq3, nq, xc3, ncc in (("ia", xi, 8, xt, 6),
                                         ("ta", xt, 6, xi, 8)):
                s = sb[m]
                d = mods[m]
                qT = projT(s["wq"], xq3, nq, s["bq"], "q")
                kT = projT(s["wk"], xc3, ncc, s["bk"], "k")
                vN = vproj_nat(xc3, s["wv"], ncc, s["bvr"], "v")

                # scores S^T per j-tile, exp, band-mask
                eTm = epool.tile([128, NT * N], CD, tag="eTm")
                for jt in range(NT):
                    st = ps_st.tile([128, N], f32, tag="st")
                    nc.tensor.matmul(st[:], kT[:, jt * 128:(jt + 1) * 128],
                                     qT[:], start=True, stop=True)
                    eT = epool.tile([128, N], CD, tag="eT")
                    nc.scalar.activation(eT[:], st[:], Exp)
                    nc.vector.tensor_mul(eTm[:, jt * N:(jt + 1) * N], eT[:],
                                         mM[:, jt * N:(jt + 1) * N])
                # virtual pad column (score q.kb, weight n_pad)
                sp = ps_small.tile([1, N], f32, tag="smallp")
                nc.tensor.matmul(sp[:], s["kbc"][:], qT[:], start=True,
                                 stop=True)
                eP = small.tile([1, N], CD, tag="eP")
                nc.scalar.activation(eP[:], sp[:], Exp)
                ePm = small.tile([1, N], CD, tag="ePm")
                nc.vector.tensor_mul(ePm[:], eP[:], mP[:])

                # softmax denominators
                ssum = ps_small.tile([1, N], f32, tag="smallp2")
                for jt in range(NT):
                    nc.tensor.matmul(ssum[:], ones_col[:],
                                     eTm[:, jt * N:(jt + 1) * N],
                                     start=(jt == 0), stop=False)
                nc.tensor.matmul(ssum[:], ones_col[0:1, :], ePm[:],
                                 start=False, stop=True)
                rinv = small.tile([1, N], CD, tag="rinv")
                with nc.allow_low_precision(
                        reason="softmax 1/denom feeds a CD-dtype matmul; "
                               "CD is >= fp16 and denom is O(1-100)"):
                    nc.vector.reciprocal(rinv[:], ssum[:])

                # O^T = V^T E^T (+ pad)
                oT = ps_acc.tile([128, N], f32, tag="acc")
                for jt in range(NT):
                    nc.tensor.matmul(oT[:], vN[:, jt * DH:(jt + 1) * DH],
                                     eTm[:, jt * N:(jt + 1) * N],
                                     start=(jt == 0), stop=False)
                nc.tensor.matmul(oT[:], s["vbp"][:], ePm[:], start=False,
                                 stop=True)

                # normalize: broadcast rinv to 128 partitions via PE
                rbc = ps_acc.tile([128, N], f32, tag="acc")
                nc.tensor.matmul(rbc[:], ones_row[:], rinv[:], start=True,
                                 stop=True)
                rbc_sb = work.tile([128, N], f32, tag="rbc_sb")
                nc.vector.tensor_copy(rbc_sb[:], rbc[:])
                onorm = work.tile([128, N], CD, tag="onorm")
                nc.vector.tensor_mul(onorm[:], oT[:], rbc_sb[:])

                # o-projection partial: po[dt*128:, :] = wo_h[:, dt].T @ onorm
                for dt_i in range(DM // 128):
                    po = ps_acc.tile([128, N], f32, tag="acc")
                    nc.tensor.matmul(po[:],
                                     s["wo"][:, dt_i * 128:(dt_i + 1) * 128],
                                     onorm[:], start=True, stop=True)
                    po_sb = work.tile([128, N], f32, tag="po_sb")
                    nc.vector.tensor_copy(po_sb[:], po[:])
                    nc.sync.dma_start(d["po"][dt_i * 128:(dt_i + 1) * 128, :],
                                      po_sb[:])

            # residual D-slices (fp32 out)
            residT(rwi, xi, 8, bxi, mods["ia"]["xr"], "xri")
            residT(rwt, xt, 6, bxt, mods["ta"]["xr"], "xrt")

    nc.compile()
    return nc


def _masks(cd):
    i = np.arange(N)
    j = np.arange(N)
    band = (j[:, None] >= i[None, :] - WINDOW // 2) & \
           (j[:, None] <= i[None, :] + WINDOW // 2 + 1)   # [j, i]
    length = band.sum(axis=0)
    npad = np.maximum(0, WINDOW - length)
    mM = band.astype(np.float64).reshape(NT, 128, N).transpose(1, 0, 2) \
             .reshape(128, NT * N)
    mP = npad.astype(np.float64)[None, :]
    return _host_cd(mM, cd), _host_cd(mP, cd)


def kernel(**inputs):
    from concourse.bass_utils import run_bass_kernel_spmd

    cd = COMPUTE_DTYPE
    if cd not in _prog_cache:
        _prog_cache[cd] = _build_program(cd)
    nc = _prog_cache[cd]

    f8 = lambda x: np.asarray(x, dtype=np.float64)
    images = f8(inputs["images"])[0]        # [N, 1024]
    caps = f8(inputs["capitions"])[0]       # [N, 768]
    ip_w, ip_b = f8(inputs["ip_w"]), f8(inputs["ip_b"])
    tp_w, tp_b = f8(inputs["tp_w"]), f8(inputs["tp_b"])

    sc = 1.0 / math.sqrt(DH)
    mM, mP = _masks(cd)
    xTi = _host_cd(images.T, cd)
    xTt = _host_cd(caps.T, cd)

    in_maps = []
    for h in range(NCORES):
        sl = slice(h * DH, (h + 1) * DH)
        im = {
            "xT_img": xTi, "xT_txt": xTt, "maskM": mM, "maskP": mP,
            "rwT_img": _host_cd(ip_w[sl].T, cd),
            "rwT_txt": _host_cd(tp_w[sl].T, cd),
            "ones_c": _host_cd(np.ones((128, 1)), cd),
            "ones_r": _host_cd(np.ones((1, 128)), cd),
            "brx": np.ascontiguousarray(ip_b[sl, None], dtype=np.float32),
            "brt": np.ascontiguousarray(tp_b[sl, None], dtype=np.float32),
        }
        for m, pw, pb, cw, cb in (("ia", ip_w, ip_b, tp_w, tp_b),
                                  ("ta", tp_w, tp_b, ip_w, ip_b)):
            qw, qb = f8(inputs[f"{m}_qw"]), f8(inputs[f"{m}_qb"])
            kw, kb = f8(inputs[f"{m}_kw"]), f8(inputs[f"{m}_kb"])
            vw, vb = f8(inputs[f"{m}_vw"]), f8(inputs[f"{m}_vb"])
            ow = f8(inputs[f"{m}_ow"])
            im[f"wqT_{m}"] = _host_cd(((qw[sl] @ pw) * sc).T, cd)
            im[f"bq_{m}"] = ((qw[sl] @ pb + qb[sl]) * sc)[:, None] \
                .astype(np.float32)
            im[f"wkT_{m}"] = _host_cd((kw[sl] @ cw).T, cd)
            im[f"bk_{m}"] = (kw[sl] @ cb + kb[sl])[:, None].astype(np.float32)
            im[f"wvT_{m}"] = _host_cd((vw[sl] @ cw).T, cd)
            im[f"bvrow_{m}"] = _host_cd((vw[sl] @ cb + vb[sl])[None, :], cd)
            im[f"woT_{m}"] = _host_cd(ow[:, sl].T, cd)
            im[f"kbcol_{m}"] = _host_cd(kb[sl, None], cd)
            im[f"vbpad_{m}"] = _host_cd(vb[sl][None, :], cd)
        in_maps.append(im)

    res = run_bass_kernel_spmd(nc, in_maps, list(range(NCORES)))
    LAST_RESULT["res"] = res

    outs = []
    for m in ("ia", "ta"):
        acc = np.zeros((DM, N), dtype=np.float64)
        for h in range(NCORES):
            r = res.results[h]
            acc += r[f"po_{m}"].astype(np.float64)
            acc[h * DH:(h + 1) * DH] += r[f"xr_{m}"].astype(np.float64)
        acc += f8(inputs["ia_ob" if m == "ia" else "ta_ob"])[:, None]
        outs.append(np.ascontiguousarray(acc.T[None]).astype(np.float32))
    return outs[0], outs[1]



# revision 15
# speedup vs baseline: 1.7398x; 1.7398x over previous
"""Trainium2 Bass kernel for nn_CrossModalFusionModel (sliding-window
cross-attention, image<->text, 2 modules).

Sharding: head-parallel over 8 cores (head h = dh 128 slice of both modules;
input/attention projection chains folded host-side into one matmul per q/k/v).
Each core emits a full-D o-projection partial (summed on host) plus its
D-slice of the residual input projection. No collectives.

v1 rewrite for the TimelineSim cost model:
- fp8e4 + DoubleRow matmuls (0.5 cyc/row, 256-deep contraction) for the q/k/v
  and residual projections; weights pre-scaled by powers of 2 into fp8's
  normal range, descaled at the psum->sbuf copy. Residual uses hi+lo fp8
  splitting of x for accuracy. o-projection in f16.
- Banded attention: per 128-query tile only the 193-wide context band is
  scored (mid/lo/hi chunks placed at matching psum partition offsets so AV
  can contract against aligned slices of natural-layout V).
- V computed directly in natural [j, dh] layout via x-stationary DoubleRow
  matmuls (no transposes).
- Zero-pad slots collapse into one virtual pad row (weight n_pad(i)) carried
  in the same "edge" psum bank as the lo/hi chunks; the pad/edge masks and
  the softmax normalization multiply e-tiles (not the o-projection).
- All wire tensors f16/fp8, packed partition-major so every DMA descriptor
  is >= 512B.
"""

import math

import numpy as np
import ml_dtypes

N = 512          # tokens / patches
DM = 1024        # d_model
DH = 128         # head dim per core
NT = N // 128    # 4 query tiles
C_IMG = 1024
C_TXT = 768
WINDOW = 64
NCORES = 8

# weight pre-scale exponents (host multiplies by 2^S, kernel divides back)
SQ = 8   # folded q weights (incl 1/sqrt(dh))
SK = 5
SV = 5
SR = 4   # residual input-proj weights

F8 = ml_dtypes.float8_e4m3
F16 = np.float16

_prog_cache = {}
LAST_RESULT = {}


def _build_program():
    import concourse.bass as bass
    import concourse.tile as tile
    from concourse import bacc, mybir

    f32 = mybir.dt.float32
    f16 = mybir.dt.float16
    f8 = mybir.dt.float8e4
    Exp = mybir.ActivationFunctionType.Exp
    Ident = mybir.ActivationFunctionType.Identity
    DR = mybir.MatmulPerfMode.DoubleRow
    Mult = mybir.AluOpType.mult
    Add = mybir.AluOpType.add

    nc = bacc.Bacc("TRN2", target_bir_lowering=False, debug=False,
                   num_devices=NCORES)

    def din(name, shape, dt):
        return nc.dram_tensor(name, shape, dt, kind="ExternalInput")

    def dout(name, shape, dt):
        return nc.dram_tensor(name, shape, dt, kind="ExternalOutput")

    # fp8 weights, packed back-to-back in ct-tile dim:
    # wq_ia(8) wk_ia(6) wv_ia(6) | wq_ta(6) wk_ta(8) wv_ta(8) = 42 tiles
    w8d = din("w8", [128, 42, DH], f8)
    xhid = din("xhi", [128, 14, N], f8)     # img^T(8) + txt^T(6), fp8 hi
    xlod = din("xlo", [128, 14, N], f8)     # fp8 lo residues (residual only)
    rw8d = din("rw8", [128, 28, DH], f8)    # ip/tp D-slices, hi(14)+lo(14)
    # f16 big block: mask canvas W(896) | wo_ia(1024) | wo_ta(1024) | id(128)
    big16d = din("big16", [128, 3072], f16)
    # f16 rows: dsc_row(128) | vb_ia(128) | vb_ta(128) | mP(512)
    row16d = din("row16", [1, 896], f16)
    # f16 cols: kb_ia | kb_ta | ones_c
    col16d = din("col16", [128, 3], f16)
    # f32 cols: bq_ia bk_ia bq_ta bk_ta br_i br_t bv_ia bv_ta (bv x 2^SV)
    colfd = din("colf", [128, 8], f32)

    po_d = {m: dout(f"po_{m}", [128, 2, 4, N], f16) for m in ("ia", "ta")}
    xr_d = {m: dout(f"xr_{m}", [DH, N], f16) for m in ("ia", "ta")}

    WOFF = {"ia": (0, 8, 14), "ta": (20, 26, 34)}   # q, k, v tile offsets
    NQ = {"ia": 8, "ta": 6}
    NC_ = {"ia": 6, "ta": 8}
    XOFF = {"ia": (0, 8), "ta": (8, 0)}             # (q-side, ctx-side) x off
    MODS = ("ia", "ta")
    BAND = [(max(0, jt * 128 - 33), min(N, jt * 128 + 160))
            for jt in range(NT)]

    with tile.TileContext(nc) as tc:
        with tc.tile_pool(name="consts", bufs=1) as consts, \
             tc.tile_pool(name="work", bufs=2) as work, \
             tc.tile_pool(name="epool", bufs=1) as epool, \
             tc.tile_pool(name="small", bufs=2) as small, \
             tc.tile_pool(name="ps_proj", bufs=2, space="PSUM") as ps_proj, \
             tc.tile_pool(name="ps_sc", bufs=2, space="PSUM") as ps_sc, \
             tc.tile_pool(name="ps_sm", bufs=2, space="PSUM") as ps_sm, \
             tc.tile_pool(name="ps_po", bufs=2, space="PSUM") as ps_po:

            # --- loads: tiny first, then critical-path order, tail last ---
            colf = consts.tile([128, 8], f32, tag="colf")
            nc.sync.dma_start(colf[:], colfd[:])
            col16 = consts.tile([128, 3], f16, tag="col16")
            nc.sync.dma_start(col16[:], col16d[:])
            row16 = consts.tile([1, 896], f16, tag="row16")
            nc.sync.dma_start(row16[:], row16d[:])
            w8 = consts.tile([128, 42, DH], f8, tag="w8")
            nc.sync.dma_start(w8[:, 0:20, :], w8d[:, 0:20, :])
            xhi = consts.tile([128, 14, N], f8, tag="xhi")
            nc.sync.dma_start(xhi[:, 0:8, :], xhid[:, 0:8, :])
            nc.sync.dma_start(xhi[:, 8:14, :], xhid[:, 8:14, :])
            nc.sync.dma_start(w8[:, 20:42, :], w8d[:, 20:42, :])
            big16 = consts.tile([128, 3072], f16, tag="big16")
            nc.sync.dma_start(big16[:], big16d[:])
            xlo = consts.tile([128, 14, N], f8, tag="xlo")
            nc.sync.dma_start(xlo[:], xlod[:])
            rw8 = consts.tile([128, 28, DH], f8, tag="rw8")
            nc.sync.dma_start(rw8[:], rw8d[:])

            W = big16[:, 0:896]               # sliding band-mask canvas
            wo = {"ia": big16[:, 896:1920], "ta": big16[:, 1920:2944]}
            id128 = big16[:, 2944:3072]
            dscr = row16[:, 0:128]            # value 2^-SV row
            vbr = {"ia": row16[:, 128:256], "ta": row16[:, 256:384]}
            mP = row16[:, 384:896]
            kb = {"ia": col16[:, 0:1], "ta": col16[:, 1:2]}
            onc = col16[:, 2:3]
            bq = {"ia": colf[:, 0:1], "ta": colf[:, 2:3]}
            bk = {"ia": colf[:, 1:2], "ta": colf[:, 3:4]}
            brx = {"ia": colf[:, 4:5], "ta": colf[:, 5:6]}
            bvc = {"ia": colf[:, 6:7], "ta": colf[:, 7:8]}

            def proj_dr(woff, nct, xoff):
                ps = ps_proj.tile([128, N], f32, tag="proj")
                for p in range(nct // 2):
                    nc.tensor.matmul(
                        ps[:], w8[:, woff + 2 * p:woff + 2 * p + 2, :],
                        xhi[:, xoff + 2 * p:xoff + 2 * p + 2, :],
                        start=(p == 0), stop=(p == nct // 2 - 1),
                        perf_mode=DR)
                return ps

            # --- phase 1: all projections (both modules) + residuals ---
            qT, kT, vT, vN, eF, eP, rinv, rbc, o16 = ({} for _ in range(9))
            for m in MODS:
                qo, ko, vo = WOFF[m]
                xq, xc = XOFF[m]
                q_ps = proj_dr(qo, NQ[m], xq)
                qT[m] = work.tile([128, N], f16, tag=f"qT{m}", name=f"qT{m}")
                nc.vector.tensor_scalar(qT[m][:], q_ps[:], 2.0 ** -SQ,
                                        bq[m], Mult, Add)
                k_ps = proj_dr(ko, NC_[m], xc)
                kT[m] = work.tile([128, N], f16, tag=f"kT{m}", name=f"kT{m}")
                nc.vector.tensor_scalar(kT[m][:], k_ps[:], 2.0 ** -SK,
                                        bk[m], Mult, Add)
                v_ps = proj_dr(vo, NC_[m], xc)
                vT[m] = work.tile([128, N], f16, tag=f"vT{m}", name=f"vT{m}")
                nc.scalar.activation(vT[m][:], v_ps[:], Ident, bias=bvc[m])

            for m, ro, xo, nct in (("ia", 0, 0, 8), ("ta", 8, 8, 6)):
                ps = ps_proj.tile([128, N], f32, tag="proj")
                first = True
                # 3 passes: rw_hi*x_hi + rw_hi*x_lo + rw_lo*x_hi
                for wro, xt_ in ((ro, xhi), (ro, xlo), (ro + 14, xhi)):
                    for p in range(nct // 2):
                        last = (wro == ro + 14) and (p == nct // 2 - 1)
                        nc.tensor.matmul(
                            ps[:], rw8[:, wro + 2 * p:wro + 2 * p + 2, :],
                            xt_[:, xo + 2 * p:xo + 2 * p + 2, :],
                            start=first, stop=last, perf_mode=DR)
                        first = False
                xr = work.tile([128, N], f16, tag=f"xr{m}")
                nc.scalar.activation(xr[:], ps[:], Ident, bias=brx[m],
                                     scale=2.0 ** -SR)
                nc.sync.dma_start(xr_d[m][:], xr[:])

            # --- phase 2: V transpose to natural [j, dh] ---
            for m in MODS:
                vt_ps = ps_po.tile([128, NT * DH], f16, tag="po")
                for jt in range(NT):
                    nc.tensor.matmul(vt_ps[:, jt * DH:(jt + 1) * DH],
                                     vT[m][:, jt * 128:(jt + 1) * 128],
                                     id128, start=True, stop=True,
                                     is_transpose=True)
                vN[m] = work.tile([128, NT * DH], f16, tag=f"vN{m}", name=f"vN{m}")
                nc.scalar.activation(vN[m][:], vt_ps[:], Ident)

            # --- phase 3: pad scores (ahead of dense, rides act queue 1st)
            for m in MODS:
                pad_ps = ps_sm.tile([1, N], f32, tag="sm")
                nc.tensor.matmul(pad_ps[:], kb[m], qT[m][:], start=True,
                                 stop=True)
                eP[m] = small.tile([1, N], f16, tag=f"eP{m}", name=f"eP{m}")
                nc.scalar.activation(eP[m][:], pad_ps[:], Exp)
                nc.vector.tensor_tensor(eP[m][:], eP[m][:], mP[:, :], Mult)

            # --- phase 4: dense scores -> exp -> band-mask ---
            for m in MODS:
                eF[m] = epool.tile([128, NT * N], f16, tag=f"eF{m}", name=f"eF{m}")
                for jt in range(NT):
                    st = ps_sc.tile([128, N], f32, tag="sc")
                    nc.tensor.matmul(st[:],
                                     kT[m][:, jt * 128:(jt + 1) * 128],
                                     qT[m][:], start=True, stop=True)
                    lo, hi = BAND[jt]
                    co = slice(jt * N + lo, jt * N + hi)
                    nc.scalar.activation(eF[m][:, co], st[:, lo:hi], Exp)
                    eng = nc.vector if jt % 2 == 0 else nc.gpsimd
                    eng.tensor_tensor(
                        eF[m][:, co], eF[m][:, co],
                        W[:, 384 - 128 * jt + lo:384 - 128 * jt + hi],
                        Mult)

            # --- phase 5: denominators, reciprocal, descale broadcast ---
            for m in MODS:
                ssum = ps_sm.tile([1, N], f32, tag="sm")
                nc.tensor.matmul(ssum[:], onc[0:1, :], eP[m][:],
                                 start=True, stop=False,
                                 skip_group_check=True)
                for jt in range(NT):
                    lo, hi = BAND[jt]
                    nc.tensor.matmul(ssum[:, lo:hi], onc[:, :],
                                     eF[m][:, jt * N + lo:jt * N + hi],
                                     start=False, stop=(jt == NT - 1),
                                     skip_group_check=True)
                rinv[m] = small.tile([1, N], f16, tag=f"ri{m}", name=f"ri{m}")
                with nc.allow_low_precision(reason="softmax 1/denom; "
                                            "denom is O(1-100), f16 ok"):
                    nc.vector.reciprocal(rinv[m][:], ssum[:])

            # --- phase 6: AV (oT keeps 2^SV) ---
            for m in MODS:
                oT = ps_po.tile([128, N], f32, tag="po")
                nc.tensor.matmul(oT[:], vbr[m], eP[m][:], start=True,
                                 stop=False, skip_group_check=True)
                for jt in range(NT):
                    lo, hi = BAND[jt]
                    nc.tensor.matmul(oT[:, lo:hi],
                                     vN[m][:, jt * DH:(jt + 1) * DH],
                                     eF[m][:, jt * N + lo:jt * N + hi],
                                     start=False, stop=(jt == NT - 1),
                                     skip_group_check=True)
                o16[m] = (oT, None)

            # --- phase 7: normalize+descale broadcast, o16 = oT * rbc ---
            for m in MODS:
                rbc_ps = ps_sm.tile([128, N], f32, tag="sm")
                nc.tensor.matmul(rbc_ps[:], dscr, rinv[m][:], start=True,
                                 stop=True)
                rbc[m] = work.tile([128, N], f32, tag=f"rb{m}", name=f"rb{m}")
                nc.vector.tensor_copy(rbc[m][:], rbc_ps[:])
                oT = o16[m][0]
                ot16 = work.tile([128, N], f16, tag=f"o{m}")
                nc.vector.tensor_tensor(ot16[:], oT[:], rbc[m][:], Mult)
                o16[m] = ot16

            # --- phase 8: o-projection partials, batched stores ---
            for m in MODS:
                for half in range(2):
                    ob = work.tile([128, 4, N], f16, tag=f"ob{m}")
                    for k4 in range(4):
                        dt_i = half * 4 + k4
                        po_ps = ps_po.tile([128, N], f32, tag="po")
                        nc.tensor.matmul(
                            po_ps[:],
                            wo[m][:, dt_i * 128:(dt_i + 1) * 128],
                            o16[m][:], start=True, stop=True)
                        eng = (nc.scalar, nc.vector, nc.scalar,
                               nc.vector)[k4]
                        if eng is nc.scalar:
                            nc.scalar.activation(ob[:, k4, :], po_ps[:],
                                                 Ident)
                        else:
                            eng.tensor_copy(ob[:, k4, :], po_ps[:])
                    nc.sync.dma_start(po_d[m][:, half, :, :], ob[:])

    nc.compile()
    return nc


def _pack_ct(a):
    """[C, X] -> [128, C//128, X] partition-major chunks."""
    C = a.shape[0]
    return np.ascontiguousarray(
        a.reshape(C // 128, 128, -1).transpose(1, 0, 2))


def _rw_hilo(rwi, rwt):
    """[128, 28, DH]: hi tiles for ip|tp slices then same-scale lo tiles."""
    hs, ls = [], []
    for w in (rwi, rwt):
        s = w.T * 2.0 ** SR                      # [C, 128]
        hi = s.astype(F8)
        lo = (s - hi.astype(np.float64)).astype(F8)
        hs.append(_pack_ct(hi))
        ls.append(_pack_ct(lo))
    return np.concatenate(hs + ls, axis=1).astype(F8)


def _fp8_hilo(a):
    hi = a.astype(F8)
    lo = (a - hi.astype(np.float64)).astype(F8)
    return hi, lo


def _masks():
    """Band-mask canvas W [128, 896] (mask_jt = W[:, 384-128jt:896-128jt])
    and the n_pad row."""
    p = np.arange(128)[:, None]
    c = np.arange(896)[None, :] - 384
    W = ((p - c >= -32) & (p - c <= 33)).astype(np.float64)
    i = np.arange(N)
    length = np.minimum(N - 1, i + 33) - np.maximum(0, i - 32) + 1
    npad = np.maximum(0, WINDOW - length).astype(np.float64)
    return W.astype(F16), npad[None, :].astype(F16)


def kernel(**inputs):
    from concourse.bass_utils import run_bass_kernel_spmd

    if "prog" not in _prog_cache:
        _prog_cache["prog"] = _build_program()
    nc = _prog_cache["prog"]

    f8_ = lambda x: np.asarray(x, dtype=np.float64)
    images = f8_(inputs["images"])[0]        # [N, 1024]
    caps = f8_(inputs["capitions"])[0]       # [N, 768]
    ip_w, ip_b = f8_(inputs["ip_w"]), f8_(inputs["ip_b"])
    tp_w, tp_b = f8_(inputs["tp_w"]), f8_(inputs["tp_b"])

    sc = 1.0 / math.sqrt(DH)
    mW, mP = _masks()
    xi_hi, xi_lo = _fp8_hilo(images.T)       # [1024, N]
    xt_hi, xt_lo = _fp8_hilo(caps.T)         # [768, N]

    xhi = np.concatenate([_pack_ct(xi_hi), _pack_ct(xt_hi)], axis=1)
    xlo = np.concatenate([_pack_ct(xi_lo), _pack_ct(xt_lo)], axis=1)

    row16 = np.zeros((1, 896), F16)
    row16[0, 0:128] = 2.0 ** -SV             # descale broadcast row
    row16[0, 384:896] = mP[0]

    in_maps = []
    for h in range(NCORES):
        sl = slice(h * DH, (h + 1) * DH)
        w8 = []
        big16 = np.zeros((128, 3072), F16)
        big16[:, 0:896] = mW
        big16[:, 2944:3072] = np.eye(128, dtype=F16)
        row = row16.copy()
        col16 = np.zeros((128, 3), F16)
        col16[:, 2] = 1.0                    # ones column
        colf = np.zeros((128, 8), np.float32)
        for mi, (m, pw, pb, cw, cb) in enumerate(
                (("ia", ip_w, ip_b, tp_w, tp_b),
                 ("ta", tp_w, tp_b, ip_w, ip_b))):
            qw, qb = f8_(inputs[f"{m}_qw"]), f8_(inputs[f"{m}_qb"])
            kw, kb = f8_(inputs[f"{m}_kw"]), f8_(inputs[f"{m}_kb"])
            vw, vb = f8_(inputs[f"{m}_vw"]), f8_(inputs[f"{m}_vb"])
            ow = f8_(inputs[f"{m}_ow"])
            w8.append(_pack_ct(((qw[sl] @ pw) * sc * 2.0 ** SQ).T))
            w8.append(_pack_ct(((kw[sl] @ cw) * 2.0 ** SK).T))
            w8.append(_pack_ct(((vw[sl] @ cw) * 2.0 ** SV).T))
            big16[:, 896 + mi * 1024:896 + (mi + 1) * 1024] = \
                ow[:, sl].T.astype(F16)
            row[0, 128 + mi * 128:256 + mi * 128] = \
                ((vw[sl] @ cb + vb[sl]) * 2.0 ** SV).astype(F16)
            col16[:, mi] = (kw[sl] @ cb + kb[sl]).astype(F16)
            colf[:, 2 * mi] = ((qw[sl] @ pb + qb[sl]) * sc).astype(np.float32)
            colf[:, 2 * mi + 1] = (kw[sl] @ cb + kb[sl]).astype(np.float32)
            colf[:, 6 + mi] = ((vw[sl] @ cb + vb[sl]) * 2.0 ** SV) \
                .astype(np.float32)
        colf[:, 4] = ip_b[sl].astype(np.float32)
        colf[:, 5] = tp_b[sl].astype(np.float32)
        im = {
            "w8": np.concatenate(w8, axis=1).astype(F8),
            "xhi": xhi, "xlo": xlo,
            "rw8": _rw_hilo(ip_w[sl], tp_w[sl]),
            "big16": big16, "row16": row, "col16": col16, "colf": colf,
        }
        in_maps.append(im)

    res = run_bass_kernel_spmd(nc, in_maps, list(range(NCORES)))
    LAST_RESULT["res"] = res

    outs = []
    for m in ("ia", "ta"):
        acc = np.zeros((DM, N), dtype=np.float64)
        for h in range(NCORES):
            r = res.results[h]
            po = r[f"po_{m}"].astype(np.float64)          # [128, 2, 4, N]
            acc += po.transpose(1, 2, 0, 3).reshape(DM, N)
            acc[h * DH:(h + 1) * DH] += r[f"xr_{m}"].astype(np.float64)
        acc += f8_(inputs["ia_ob" if m == "ia" else "ta_ob"])[:, None]
        outs.append(np.ascontiguousarray(acc.T[None]).astype(np.float32))
    return outs[0], outs[1]
